# revision 56
# baseline (speedup 1.0000x reference)
"""Trainium2 Bass kernel for nn_ReasonerModel (12-layer cross-attn transformer).

Sharding: data-parallel over batch. 32 batch elems / 8 cores = 4 per core.
Each core streams the full weights (host-precast bf16/fp8, pre-tiled layouts)
and computes its 4 batch rows end-to-end; no collectives.

v3 design (on top of v2's fully-transposed layout):
  - residual stream xT in fp16 (halves DVE cost via 2x modes, makes the
    LN-stat ones-matmuls 1 cyc/row instead of fp32's 4)
  - softmax sums via DoubleRow ones(=1/64) matmuls; 1/sum via the DVE
    custom reciprocal_approx_fast; partition-broadcasts on GpSimd
    (removes all PE broadcast matmuls + the ACT Ln/Exp recip chain)
  - K/V projection loops reordered so consecutive matmuls share lhsT
    (amortizes LDWEIGHTS, which the v2 trace showed serializing at
    ~250ns/MM vs the 107ns DR streaming floor)
  - residual drains fused to single custom-DVE ops (affine_then_add)
Layouts:
  xT      [128, 8, 4, 80] f16   residual stream (d on partitions)
  hbf     [128, 8, 4, 80] bf16  bf16 cast of LN1 out, feeds the MLP
  hq8     [128, 8, 4, 80] fp8   fp8 cast of residual/LN2 out, feeds scores
  know_b  [128, 8, 1024] fp8    d-on-partitions know, resident per (b)
  kT_b    [128, 8, 1024] fp8    K^T per b (n on partitions, s free)
  vb      [128, 8, 1024] fp8    V per b (s on partitions, n free)
  wT_b    [128, 8, 16, 80] fp8  exp(scores^T) (s on partitions)
  aT      [128, 8, 4, 80] fp8   attention out (n on partitions)
  gT      [128, 32, 4, 80] bf16 gelu(fc) (4D-features on partitions)
"""

import os
import sys

sys.path.insert(0, "/opt/trn_rl_repo")

import numpy as np

import concourse.bass as bass
import concourse.tile as tile
from concourse import mybir
import concourse.bass_utils as _bu
from concourse.bass_utils import run_bass_kernel_spmd
from concourse.vector_clock import ScopedClock

if os.environ.get("KERNEL_LDW_OPT", "0") == "1":
    _orig_run_command = _bu.run_command

    def _run_command_ldw(argv, **kwargs):
        if isinstance(argv, list):
            argv = ["--enable-ldw-opt=true" if a == "--enable-ldw-opt=false"
                    else a for a in argv]
        return _orig_run_command(argv, **kwargs)

    _bu.run_command = _run_command_ldw

# model dims (fixed by the problem)
B, SQ, SKV, D, H = 32, 80, 1024, 1024, 16
L = int(os.environ.get("KERNEL_LAYERS", "12"))
HD = D // H          # 64
N_CORES = 8
BL = B // N_CORES    # 4 batch rows per core
DT = D // 128        # 8 d-tiles
FT = 4 * D // 128    # 32 ffn tiles
BQ = BL * SQ         # 320
EPS = 1e-5
SCALE = 1.0 / np.sqrt(HD)

F32 = mybir.dt.float32
F16 = mybir.dt.float16
BF16 = mybir.dt.bfloat16
FP8 = mybir.dt.float8e4
AF = mybir.ActivationFunctionType
ALU = mybir.AluOpType
FP8_SCALE = 64.0           # host prescales know + all weights into e4m3 range
FP8_INV = 1.0 / (FP8_SCALE * FP8_SCALE)
AV_SCALE = 4096.0          # aT carries 4096*a so fp8 stays in normal range
DR = mybir.MatmulPerfMode.DoubleRow


class PatchedTC(tile.TileContext):
    """This container's walrus accepts at most ONE sem wait per instruction;
    Tile may attach several. Peel extras onto preceding same-engine no-ops."""

    def _commit_instruction(self, inst, lazy_reg_writes: bool = True):
        si = getattr(inst, "sync_info", None)
        if (
            si is not None
            and si.on_wait
            and len(si.on_wait) > 1
            and inst.engine != mybir.EngineType.Unassigned
        ):
            waits = list(si.on_wait)
            si.on_wait = [waits[-1]]
            for j, w in enumerate(waits[:-1]):
                nop = mybir.InstNoOp(
                    name=f"{inst.name}-sw{j}",
                    sync_info=mybir.SyncInfo(on_wait=[w], on_update=[]),
                    bass_nofuse=True,
                    engine=inst.engine,
                )
                super()._commit_instruction(nop, lazy_reg_writes=False)
        return super()._commit_instruction(inst, lazy_reg_writes)

    def _drain_and_barrier(self, tick_clock, wait_clock):
        drain_inst = self.nc.sync.drain()
        wait_clock.add_sem_waits(
            drain_inst.ins, ScopedClock({None: tick_clock.global_clock})
        )
        si = drain_inst.ins.sync_info
        if si is not None and si.on_wait and len(si.on_wait) > 1:
            waits = list(si.on_wait)
            si.on_wait = waits[:1]
            for w in waits[1:]:
                extra = self.nc.sync.drain()
                nsi = extra.ins.sync_info
                if nsi is None:
                    extra.ins.sync_info = mybir.SyncInfo(on_wait=[w], on_update=[])
                else:
                    nsi.on_wait = [w]
        self.nc.all_engine_barrier()
        assert self.sems is not None
        popped = self.nc._tile_sem_poison_stack.pop()
        assert popped is self._sem_poison
        self.nc.clear_and_free_semaphores(list(self.sems.allocated().values()))
        self.nc.all_engine_barrier()


def bcast_ap(ap_1d, p):
    """Partition-broadcast a 1-D DRAM AP to [p, n] (stride-0 partition dim)."""
    return bass.AP(
        tensor=ap_1d.tensor, offset=ap_1d.offset, ap=[[0, p]] + list(ap_1d.ap)
    )


def build_nc():
    try:  # lift the stale 192KB/partition SBUF cap to the real usable 208KB
        from concourse import tile_utils

        tile_utils.max_sbuf_usage = 208 * 1024
    except Exception:
        pass

    nc = bass.Bass("TRN2", target_bir_lowering=False, debug=False,
                   num_devices=N_CORES)

    # ---- DRAM I/O (host-prepped layouts; see _prep() below) ----
    xT_in = nc.dram_tensor("xT0", [128, DT, BL, SQ], F16, kind="ExternalInput")
    knowT = nc.dram_tensor("knowT", [BL, 128, DT, SKV], FP8,
                           kind="ExternalInput")
    Wk = nc.dram_tensor("Wk", [L, DT, 128, DT, 128], FP8, kind="ExternalInput")
    Wv = nc.dram_tensor("Wv", [L, 128, DT, D], FP8, kind="ExternalInput")
    Wp = nc.dram_tensor("Wp", [L, DT, 128, DT, 128], FP8, kind="ExternalInput")
    Wf = nc.dram_tensor("Wf", [L, FT, 128, DT, 128], BF16, kind="ExternalInput")
    Wm = nc.dram_tensor("Wm", [L, DT, 128, FT, 128], BF16, kind="ExternalInput")
    bk = nc.dram_tensor("bk", [L, 128, DT], F32, kind="ExternalInput")
    bv = nc.dram_tensor("bv", [L, D], BF16, kind="ExternalInput")
    bp = nc.dram_tensor("bp", [L, 128, DT], F32, kind="ExternalInput")
    bf = nc.dram_tensor("bf", [L, 128, FT], F32, kind="ExternalInput")
    bm = nc.dram_tensor("bm", [L, 128, DT], F32, kind="ExternalInput")
    g1 = nc.dram_tensor("g1", [L, 128, DT], F32, kind="ExternalInput")
    b1 = nc.dram_tensor("b1", [L, 128, DT], F32, kind="ExternalInput")
    g2 = nc.dram_tensor("g2", [L, 128, DT], F32, kind="ExternalInput")
    b2 = nc.dram_tensor("b2", [L, 128, DT], F32, kind="ExternalInput")
    out_ext = nc.dram_tensor("out", [128, DT, BL, SQ], F16,
                             kind="ExternalOutput")

    with PatchedTC(nc) as tc:
        import contextlib

        ctx = contextlib.ExitStack()
        with ctx:
            P = lambda **kw: ctx.enter_context(tc.tile_pool(**kw))
            singles = P(name="singles", bufs=1)
            kv_pool = P(name="kv", bufs=2)       # kT_b + vb
            wT_pool = P(name="wT", bufs=2)
            wkv_pool = P(name="wkv", bufs=2)
            wch_pool = P(name="wch", bufs=4)     # wf chunks
            wp_pool = P(name="wpch", bufs=2)     # wp chunks
            wm_pool = P(name="wm", bufs=3)       # wm chunks (bigger)
            bc_pool = P(name="bc", bufs=2)       # broadcast tiles
            sb_pool = P(name="sb", bufs=2)       # per-layer small biases
            stA_pool = P(name="stA", bufs=1)     # LN tiny stats
            stB_pool = P(name="stB", bufs=2)     # softmax recip tiles
            sq_pool = P(name="sq", bufs=2)       # x^2 / LN scratch
            psA = P(name="psA", bufs=3, space="PSUM")  # [128,512] kv/proj/fc/mlp
            psS = P(name="psS", bufs=2, space="PSUM")  # [128,4,80] scoresT
            psV = P(name="psV", bufs=2, space="PSUM")  # [128,160] AV
            psM = P(name="psM", bufs=1, space="PSUM")  # [1,*] sums/LN stats
            drs = P(name="drs", bufs=4, space="DRAM")  # broadcast bounce bufs

            # ---- constants ----
            ones_f16 = singles.tile([128, 1], F16)
            nc.vector.memset(ones_f16, 1.0)
            # DR pair-ones carrying the 1/64 softmax-sum prescale; padded to
            # 16 cols so the pair stride is 16B (DR ldweights step%16==0).
            ones_i64 = singles.tile([128, 2, 16], FP8)
            nc.vector.memset(ones_i64, 1.0 / FP8_SCALE)
            eps_t = singles.tile([1, 1], F32)
            nc.vector.memset(eps_t, EPS)
            ones_1h = singles.tile([1, 16], F16)
            nc.vector.memset(ones_1h, 1.0)
            ones_1f = singles.tile([1, 16], F32)
            nc.vector.memset(ones_1f, 1.0)

            # ---- persistent activations ----
            xT = singles.tile([128, DT, BL, SQ], F16, tag="xT")
            nc.sync.dma_start(out=xT, in_=xT_in[:, :, :, :])
            know_res = singles.tile([128, BL, DT, SKV], FP8, tag="know")
            for kb in range(BL):
                nc.sync.dma_start(out=know_res[:, kb], in_=knowT[kb])
            # hbf holds the bf16 cast of LN1 out (MLP input); hq8 the fp8
            # cast of the residual (scores q input).
            hbf = singles.tile([128, DT, BL, SQ], BF16, tag="hbf")
            hq8 = singles.tile([128, DT, BL, SQ], FP8, tag="hq8")
            for dt in range(DT):
                nc.vector.tensor_copy(out=hq8[:, dt], in_=xT[:, dt])

            aT = singles.tile([128, DT, BL, SQ], FP8, tag="aT")
            gT = singles.tile([128, FT, BL, SQ], BF16, tag="gT")

            def ln_stats():
                """LN stats over the partition(d) axis of xT -> broadcast
                tile bc [128, 2, BQ] fp16 with bc[:,0]=mu, bc[:,1]=rstd."""
                ps_s = psM.tile([1, BQ], F32, tag="psM", name="ps_s")
                ps_q = psS.tile([1, BQ], F32, tag="psS", name="ps_q")
                for dt in range(DT):
                    x2 = xT[:, dt].rearrange("p b q -> p (b q)")
                    xsq = sq_pool.tile([128, BQ], F16, tag="lns", name="xsq")
                    nc.vector.tensor_tensor(xsq, x2, x2, ALU.mult)
                    nc.tensor.matmul(
                        ps_s, lhsT=ones_f16, rhs=x2,
                        start=(dt == 0), stop=(dt == DT - 1))
                    nc.tensor.matmul(
                        ps_q, lhsT=ones_f16, rhs=xsq,
                        start=(dt == 0), stop=(dt == DT - 1))
                stats = stA_pool.tile([1, 2, BQ], F16, tag="stats")
                # mu = ps_s/D ; musq = (ps_s/D)^2 ; var = ps_q/D - musq
                nc.vector.tensor_scalar_mul(stats[:, 0], ps_s, 1.0 / D)
                musq = stA_pool.tile([1, BQ], F32, tag="musq")
                nc.scalar.activation(musq, ps_s, AF.Square, scale=1.0 / D)
                var = stA_pool.tile([1, BQ], F32, tag="var")
                nc.vector.scalar_tensor_tensor(
                    out=var, in0=ps_q, scalar=1.0 / D, in1=musq,
                    op0=ALU.mult, op1=ALU.subtract)
                # rstd = exp(-0.5*ln(var+eps))  (Reciprocal/Rsqrt LUTs are
                # unavailable in this container's walrus)
                lnv = stA_pool.tile([1, BQ], F32, tag="lnv")
                nc.scalar.activation(lnv, var, AF.Ln, bias=eps_t)
                nc.scalar.activation(stats[:, 1], lnv, AF.Exp, scale=-0.5)
                # partition-broadcast via DRAM bounce (stride-0 read-back)
                dt_b = drs.tile([1, 2 * BQ], F16, tag="lnd")
                nc.sync.dma_start(
                    out=dt_b[0], in_=stats.rearrange("p a q -> p (a q)"))
                bc = bc_pool.tile([128, 2, BQ], F16, tag="lnbc")
                nc.gpsimd.dma_start(
                    out=bc.rearrange("p a q -> p (a q)"),
                    in_=bcast_ap(dt_b[0], 128))
                # PE keep-warm ticks pegged to the serial chain's stages so
                # the HAM window doesn't re-throttle during the stats gap
                for t, lh in ((var[:, 0:64], ones_1f[:, 0:1]),
                              (lnv[:, 0:64], ones_1f[:, 0:1]),
                              (stats[:, 0, 0:64], ones_1h[:, 0:1]),
                              (bc[0:1, 0, 0:64], ones_1h[:, 0:1])):
                    pw = psM.tile([1, 64], F32, tag="psM", name="warm")
                    nc.tensor.matmul(pw, lhsT=lh, rhs=t, start=True, stop=True)
                return bc

            def ln_apply(bc, g_sb, b_sb, cast_out):
                """x = (x - mu)*rstd*g + b ; cast_out = lowprec(x)."""
                for dt in range(DT):
                    x2 = xT[:, dt].rearrange("p b q -> p (b q)")
                    t = sq_pool.tile([128, BQ], F16, tag="lns", name="lnt")
                    nc.vector.tensor_tensor(t, x2, bc[:, 0], ALU.subtract)
                    nc.vector.tensor_tensor(t, t, bc[:, 1], ALU.mult)
                    nc.vector.tensor_scalar(
                        x2, t, g_sb[:, dt:dt + 1], b_sb[:, dt:dt + 1],
                        op0=ALU.mult, op1=ALU.add)
                    h2 = cast_out[:, dt].rearrange("p b q -> p (b q)")
                    if dt % 2 == 0:
                        nc.scalar.copy(out=h2, in_=x2)
                    else:
                        nc.vector.tensor_copy(out=h2, in_=x2)
                    if dt == 0:
                        # one keep-warm tick, ready exactly when the first
                        # consumer matmuls are (apply dt=0) — bridges the
                        # bounce->apply HAM hole without delaying them
                        pw = psM.tile([1, 64], F32, tag="psM", name="warm2")
                        nc.tensor.matmul(pw, lhsT=ones_f16, rhs=x2[:, 0:64],
                                         start=True, stop=True)

            def emit_layer_weights(l):
                """DMA layer-l kv weights + biases; returns handle dict."""
                w = {}
                w["wk"] = wkv_pool.tile([128, DT, DT, 128], FP8, tag="wk", name="wk")
                nc.sync.dma_start(
                    out=w["wk"], in_=Wk[l].rearrange("t p d n -> p t d n"))
                w["wv"] = wkv_pool.tile([128, DT, D], FP8, tag="wv", name="wv")
                nc.sync.dma_start(out=w["wv"], in_=Wv[l])
                for nm, src in [("bk", bk), ("bp", bp), ("bm", bm),
                                ("g1", g1), ("b1", b1), ("g2", g2),
                                ("b2", b2)]:
                    w[nm] = sb_pool.tile([128, DT], F32, tag=nm, name=nm)
                    nc.sync.dma_start(out=w[nm], in_=src[l])
                w["bf"] = sb_pool.tile([128, FT], F32, tag="bf", name="bfs")
                nc.sync.dma_start(out=w["bf"], in_=bf[l])
                w["bv"] = bc_pool.tile([128, D], BF16, tag="bv", name="bv")
                nc.gpsimd.dma_start(out=w["bv"], in_=bcast_ap(bv[l], 128))
                return w

            def emit_kT(w, b, nt_lo=0, nt_hi=DT, kTb=None):
                """K^T [n-part, s] for one b. k2-outer/sc-inner so the two
                MMs of each k2 share lhsT (amortizes LDWEIGHTS)."""
                if kTb is None:
                    kTb = kv_pool.tile([128, DT, SKV], FP8, tag="kT")
                for nt in range(nt_lo, nt_hi):
                    ps0 = psA.tile([128, 512], F32, tag="psA", name="psk0")
                    ps1 = psA.tile([128, 512], F32, tag="psA", name="psk1")
                    pss = (ps0, ps1)
                    for k2 in range(DT // 2):
                        for sc in range(2):
                            nc.tensor.matmul(
                                pss[sc],
                                lhsT=w["wk"][:, nt, 2 * k2:2 * k2 + 2, :],
                                rhs=know_res[:, b, 2 * k2:2 * k2 + 2,
                                             sc * 512:(sc + 1) * 512],
                                start=(k2 == 0), stop=(k2 == DT // 2 - 1),
                                perf_mode=DR)
                    # both drains on DVE: ACT is exp-saturated during the
                    # attention phase and would hold the psA slots hostage
                    for sc in range(2):
                        nc.vector.tensor_scalar(
                            kTb[:, nt, sc * 512:(sc + 1) * 512], pss[sc],
                            1.0 / FP8_SCALE, w["bk"][:, nt:nt + 1],
                            op0=ALU.mult, op1=ALU.add)
                return kTb

            def emit_V(w, b):
                """V [s-part, n] for one b. k2-outer/nh-inner so the two MMs
                of each k2 share lhsT (know chunk)."""
                vb = kv_pool.tile([128, DT, D], FP8, tag="v")
                for sv in range(DT):
                    ps0 = psA.tile([128, 512], F32, tag="psA", name="psv0")
                    ps1 = psA.tile([128, 512], F32, tag="psA", name="psv1")
                    pss = (ps0, ps1)
                    for k2 in range(DT // 2):
                        for nh in range(2):
                            nc.tensor.matmul(
                                pss[nh],
                                lhsT=know_res[:, b, 2 * k2:2 * k2 + 2,
                                              sv * 128:(sv + 1) * 128],
                                rhs=w["wv"][:, 2 * k2:2 * k2 + 2,
                                            nh * 512:(nh + 1) * 512],
                                start=(k2 == 0), stop=(k2 == DT // 2 - 1),
                                perf_mode=DR)
                    for nh in range(2):
                        nc.vector.scalar_tensor_tensor(
                            out=vb[:, sv, nh * 512:(nh + 1) * 512],
                            in0=pss[nh], scalar=1.0 / FP8_SCALE,
                            in1=w["bv"][:, nh * 512:(nh + 1) * 512],
                            op0=ALU.mult, op1=ALU.add)
                return vb

            def emit_scores(b, kTb):
                """scores^T -> exp into wT_b [s-part, sc, h, q]."""
                wTb = wT_pool.tile([128, DT, H, SQ], FP8, tag="wT")
                for h in range(H):
                    po = (h % 2) * 64
                    hp = h // 2
                    for g in range(2):
                        ps = psS.tile([128, 4, SQ], F32, tag="psS", name="pss")
                        for j in range(4):
                            sc = g * 4 + j
                            nc.tensor.matmul(
                                ps[:, j, :],
                                lhsT=kTb[po:po + 64, hp,
                                         sc * 128:(sc + 1) * 128],
                                rhs=hq8[po:po + 64, hp, b, :],
                                start=True, stop=True)
                        nc.scalar.activation(
                            out=wTb[:, g * 4:(g + 1) * 4, h, :],
                            in_=ps, func=AF.Exp, scale=SCALE / FP8_SCALE)
                return wTb

            def emit_sums(b, wTb):
                """softmax sums via DR ones(1/64) -> 64/sum via DVE fast
                reciprocal -> GpSimd partition-broadcast into rs_sb (f32)."""
                rs_sb = bc_pool.tile([128, H, SQ], F16, tag="rssb")
                for hg in range(4):
                    if hg % 2 == 0:
                        ps = psM.tile([1, 4 * SQ], F32, tag="psM",
                                      name="pssum")
                    else:
                        ps = psS.tile([1, 4 * SQ], F32, tag="psS",
                                      name="pssum2")
                    for g in range(4):
                        nc.tensor.matmul(
                            ps, lhsT=ones_i64[:, :, 0:1],
                            rhs=wTb[:, 2 * g:2 * g + 2, hg * 4:(hg + 1) * 4, :]
                            .rearrange("p k h q -> p k (h q)"),
                            start=(g == 0), stop=(g == 3),
                            perf_mode=DR)
                    # rs = 1/ps = 64/sum (ones carry 1/64): exp(-ln(ps))
                    lnp = stB_pool.tile([1, 4 * SQ], F32, tag="lnp")
                    nc.scalar.activation(lnp, ps, AF.Ln)
                    rp = stB_pool.tile([1, 4 * SQ], F16, tag="rs")
                    nc.scalar.activation(rp, lnp, AF.Exp, scale=-1.0)
                    dt_b = drs.tile([1, 4 * SQ], F16, tag="rsd")
                    nc.sync.dma_start(out=dt_b[0], in_=rp)
                    nc.gpsimd.dma_start(
                        out=rs_sb[:, hg * 4:(hg + 1) * 4, :]
                        .rearrange("p h q -> p (h q)"),
                        in_=bcast_ap(dt_b[0], 128))
                return rs_sb

            def emit_AV(b, wTb, vb, rs_sb):
                """AV (head pairs) + normalize into aT[:, :, b, :]."""
                for hp in range(DT):
                    ps = psV.tile([128, 2 * SQ], F32, tag="psV", name="psav")
                    for sv in range(DT):
                        nc.tensor.matmul(
                            ps,
                            lhsT=vb[:, sv, hp * 128:(hp + 1) * 128],
                            rhs=wTb[:, sv, 2 * hp:2 * hp + 2, :].rearrange(
                                "p h q -> p (h q)"),
                            start=(sv == 0), stop=(sv == DT - 1))
                    nc.vector.tensor_tensor(
                        aT[0:64, hp, b, :], ps[0:64, 0:SQ],
                        rs_sb[0:64, 2 * hp, :], ALU.mult)
                    nc.vector.tensor_tensor(
                        aT[64:128, hp, b, :], ps[64:128, SQ:2 * SQ],
                        rs_sb[64:128, 2 * hp + 1, :], ALU.mult)

            # ================= layers (kv software-pipelined) =================
            # Emission order keeps independent matmul work queued ahead of
            # dependent ones (PE queue is strict FIFO: a waiting MM blocks
            # ready MMs behind it): V(b+1) lands before sums(b) so exp(b) has
            # drained; scores(b+1) lands before AV(b) so the rs broadcast
            # bounce has landed.
            w = emit_layer_weights(0)
            kT_t = {0: emit_kT(w, 0)}
            v_t = {0: emit_V(w, 0)}
            for l in range(L):
                wn = emit_layer_weights(l + 1) if l + 1 < L else None
                wTb = {0: emit_scores(0, kT_t[0])}
                if 1 < BL:
                    kT_t[1] = emit_kT(w, 1)
                kTb_n = None
                for b in range(BL):
                    if b + 1 < BL:
                        v_t[b + 1] = emit_V(w, b + 1)
                    rs_sb = emit_sums(b, wTb[b])
                    if b + 1 < BL:
                        wTb[b + 1] = emit_scores(b + 1, kT_t[b + 1])
                    if b + 2 < BL:
                        kT_t[b + 2] = emit_kT(w, b + 2)
                    elif b == BL - 1 and wn is not None:
                        # fills the sums(3) -> rs-bounce -> AV(3) latency
                        kTb_n = emit_kT(wn, 0, 0, DT // 2)
                    emit_AV(b, wTb[b], v_t[b], rs_sb)

                # 5-deep psum rotation for the dense-GEMM phases: psA's 3
                # slots plus psS's 2 (idle outside attention) so a group's
                # drain lag never stalls the next group's matmuls.
                def mm_ps(i, name):
                    j = i % 7
                    if j < 3:
                        return psA.tile([128, 512], F32, tag="psA", name=name)
                    if j < 5:
                        return psS.tile([128, 512], F32, tag="psS", name=name)
                    return psV.tile([128, 512], F32, tag="psV", name=name)

                # ---- attention out-projection + residual (fused drain) ----
                for nt in range(DT):
                    wpc = wp_pool.tile([128, DT, 128], FP8, tag="wp")
                    nc.sync.dma_start(out=wpc, in_=Wp[l, nt])
                    ps = mm_ps(nt, "psp")
                    for k2 in range(DT // 2):
                        nc.tensor.matmul(
                            ps[:, :BQ], lhsT=wpc[:, 2 * k2:2 * k2 + 2, :],
                            rhs=aT[:, 2 * k2:2 * k2 + 2].rearrange(
                                "p d b q -> p d (b q)"),
                            start=(k2 == 0), stop=(k2 == DT // 2 - 1),
                            perf_mode=DR)
                    x2 = xT[:, nt].rearrange("p b q -> p (b q)")
                    t = sq_pool.tile([128, BQ], F16, tag="lns", name="prt")
                    nc.vector.tensor_scalar(
                        t, ps[:, :BQ], 1.0 / (AV_SCALE * FP8_SCALE),
                        w["bp"][:, nt:nt + 1], op0=ALU.mult, op1=ALU.add)
                    nc.vector.tensor_tensor(x2, x2, t, ALU.add)

                # ---- LN1 stats, then next layer's V (fills the gap) ----
                bc1 = ln_stats()
                if l + 1 < L:
                    vb_n = emit_V(wn, 0)
                g1s, b1s, g2s, b2s, bfs = (w["g1"], w["b1"], w["g2"],
                                           w["b2"], w["bf"])
                ln_apply(bc1, g1s, b1s, hbf)

                # ---- ffn in + gelu ----
                for nt in range(FT):
                    wfc = wch_pool.tile([128, DT, 128], BF16, tag="wf")
                    nc.sync.dma_start(out=wfc, in_=Wf[l, nt])
                    ps = mm_ps(nt, "psf")
                    for kt in range(DT):
                        nc.tensor.matmul(
                            ps[:, :BQ], lhsT=wfc[:, kt],
                            rhs=hbf[:, kt].rearrange("p b q -> p (b q)"),
                            start=(kt == 0), stop=(kt == DT - 1))
                    nc.scalar.activation(
                        out=gT[:, nt].rearrange("p b q -> p (b q)"),
                        in_=ps[:, :BQ], func=AF.Gelu_apprx_tanh,
                        bias=bfs[:, nt:nt + 1])

                # ---- ffn out + residual ----
                for nt in range(DT):
                    wmc = wm_pool.tile([128, FT, 128], BF16, tag="wm")
                    nc.sync.dma_start(out=wmc, in_=Wm[l, nt])
                    ps = mm_ps(nt, "psm")
                    for kt in range(FT):
                        nc.tensor.matmul(
                            ps[:, :BQ], lhsT=wmc[:, kt],
                            rhs=gT[:, kt].rearrange("p b q -> p (b q)"),
                            start=(kt == 0), stop=(kt == FT - 1))
                    nc.vector.scalar_tensor_tensor(
                        out=xT[:, nt].rearrange("p b q -> p (b q)"),
                        in0=ps[:, :BQ], scalar=w["bm"][:, nt:nt + 1],
                        in1=xT[:, nt].rearrange("p b q -> p (b q)"),
                        op0=ALU.add, op1=ALU.add)

                # ---- LN2 stats; finish next layer's kT (fills the gap) ----
                bc2 = ln_stats()
                if l + 1 < L:
                    emit_kT(wn, 0, DT // 2, DT, kTb=kTb_n)
                    kT_t, v_t, w = {0: kTb_n}, {0: vb_n}, wn
                ln_apply(bc2, g2s, b2s, hq8)

            # epilogue: residual out (transposed; host un-transposes)
            nc.sync.dma_start(out=out_ext[:, :, :, :], in_=xT)

    return nc


_CACHE = {}


def _prep(inputs):
    """Host-side layout/dtype prep. Returns per-core in_maps."""
    import ml_dtypes

    bf16 = ml_dtypes.bfloat16
    fp8 = ml_dtypes.float8_e4m3
    f32 = np.float32

    x = np.asarray(inputs["input_ids"], f32) + np.asarray(
        inputs["pos_embed"], f32)[None]
    know = np.asarray(inputs["input_ids_know"], f32)
    Wa = np.asarray(inputs["W_attn"], f32)
    ba = np.asarray(inputs["b_attn"], f32)
    Wpm = np.asarray(inputs["W_proj_attn"], f32)
    bpm = np.asarray(inputs["b_proj_attn"], f32)
    Wfm = np.asarray(inputs["W_fc"], f32)
    bfm = np.asarray(inputs["b_fc"], f32)
    Wmm = np.asarray(inputs["W_proj_mlp"], f32)
    bmm = np.asarray(inputs["b_proj_mlp"], f32)

    def pt(a):  # [L, D'] -> [L, 128, T] with element [l, p, t] = a[l, t*128+p]
        return np.ascontiguousarray(
            a.reshape(L, -1, 128).transpose(0, 2, 1), f32)

    shared = {
        # Wk[l, nt, p, dt, n] = Wa[l, dt*128+p, D + nt*128+n]  (x64, fp8)
        "Wk": np.ascontiguousarray(
            (Wa[:L, :, D:2 * D] * 64.0).reshape(L, DT, 128, DT, 128)
            .transpose(0, 3, 2, 1, 4).astype(fp8)),
        # Wv[l, p, dt, n] = Wa[l, dt*128+p, 2D + n]  (x64, fp8)
        "Wv": np.ascontiguousarray(
            (Wa[:L, :, 2 * D:] * 64.0).reshape(L, DT, 128, D)
            .transpose(0, 2, 1, 3).astype(fp8)),
        # Wp[l, nt, p, kt, n] = Wp[l, kt*128+p, nt*128+n]  (x64, fp8)
        "Wp": np.ascontiguousarray(
            (Wpm[:L] * 64.0).reshape(L, DT, 128, DT, 128)
            .transpose(0, 3, 2, 1, 4).astype(fp8)),
        "Wf": np.ascontiguousarray(
            Wfm[:L].reshape(L, DT, 128, FT, 128)
            .transpose(0, 3, 2, 1, 4).astype(bf16)),
        "Wm": np.ascontiguousarray(
            Wmm[:L].reshape(L, FT, 128, DT, 128)
            .transpose(0, 3, 2, 1, 4).astype(bf16)),
        "bk": pt(ba[:L, D:2 * D] * 64.0),
        "bv": np.ascontiguousarray((ba[:L, 2 * D:] * 64.0).astype(bf16)),
        "bp": pt(bpm[:L]),
        "bf": pt(bfm[:L]),
        "bm": pt(bmm[:L]),
        "g1": pt(np.asarray(inputs["ln1_g"], f32)[:L]),
        "b1": pt(np.asarray(inputs["ln1_b"], f32)[:L]),
        "g2": pt(np.asarray(inputs["ln2_g"], f32)[:L]),
        "b2": pt(np.asarray(inputs["ln2_b"], f32)[:L]),
    }

    in_maps = []
    for c in range(N_CORES):
        m = dict(shared)
        xs = x[c * BL:(c + 1) * BL]  # [BL, SQ, D]
        # xT[p, dt, b, q] = xs[b, q, dt*128+p]
        m["xT0"] = np.ascontiguousarray(
            xs.reshape(BL, SQ, DT, 128).transpose(3, 2, 0, 1)).astype(
                np.float16)
        ks = know[c * BL:(c + 1) * BL]  # [BL, SKV, D]
        # knowT[b, p, dt, s] = ks[b, s, dt*128+p]
        m["knowT"] = np.ascontiguousarray(
            (ks * 64.0).reshape(BL, SKV, DT, 128)
            .transpose(0, 3, 2, 1).astype(fp8))
        in_maps.append(m)
    return in_maps


def kernel(**inputs):
    if "nc" not in _CACHE:
        _CACHE["nc"] = build_nc()
    nc = _CACHE["nc"]

    in_maps = _prep(inputs)
    _CACHE["last_in_maps"] = in_maps

    res = run_bass_kernel_spmd(nc, in_maps, list(range(N_CORES)))
    outs = []
    for c in range(N_CORES):
        oT = np.asarray(res.results[c]["out"]).astype(np.float32)
        # out[b, q, dt*128+p] = oT[p, dt, b, q]
        outs.append(oT.transpose(2, 3, 1, 0).reshape(BL, SQ, D))
    return np.ascontiguousarray(np.concatenate(outs, axis=0), np.float32)


# revision 58
# speedup vs baseline: 1.0140x; 1.0140x over previous
"""Trainium2 Bass kernel for nn_ReasonerModel (12-layer cross-attn transformer).

Sharding: data-parallel over batch. 32 batch elems / 8 cores = 4 per core.
Each core streams the full weights (host-precast bf16/fp8, pre-tiled layouts)
and computes its 4 batch rows end-to-end; no collectives.

v3 design (on top of v2's fully-transposed layout):
  - residual stream xT in fp16 (halves DVE cost via 2x modes, makes the
    LN-stat ones-matmuls 1 cyc/row instead of fp32's 4)
  - softmax sums via DoubleRow ones(=1/64) matmuls; 1/sum via the DVE
    custom reciprocal_approx_fast; partition-broadcasts on GpSimd
    (removes all PE broadcast matmuls + the ACT Ln/Exp recip chain)
  - K/V projection loops reordered so consecutive matmuls share lhsT
    (amortizes LDWEIGHTS, which the v2 trace showed serializing at
    ~250ns/MM vs the 107ns DR streaming floor)
  - residual drains fused to single custom-DVE ops (affine_then_add)
Layouts:
  xT      [128, 8, 4, 80] f16   residual stream (d on partitions)
  hbf     [128, 8, 4, 80] bf16  bf16 cast of LN1 out, feeds the MLP
  hq8     [128, 8, 4, 80] fp8   fp8 cast of residual/LN2 out, feeds scores
  know_b  [128, 8, 1024] fp8    d-on-partitions know, resident per (b)
  kT_b    [128, 8, 1024] fp8    K^T per b (n on partitions, s free)
  vb      [128, 8, 1024] fp8    V per b (s on partitions, n free)
  wT_b    [128, 8, 16, 80] fp8  exp(scores^T) (s on partitions)
  aT      [128, 8, 4, 80] fp8   attention out (n on partitions)
  gT      [128, 32, 4, 80] bf16 gelu(fc) (4D-features on partitions)
"""

import os
import sys

sys.path.insert(0, "/opt/trn_rl_repo")

import numpy as np

import concourse.bass as bass
import concourse.tile as tile
from concourse import mybir
import concourse.bass_utils as _bu
from concourse.bass_utils import run_bass_kernel_spmd
from concourse.vector_clock import ScopedClock

if os.environ.get("KERNEL_LDW_OPT", "0") == "1":
    _orig_run_command = _bu.run_command

    def _run_command_ldw(argv, **kwargs):
        if isinstance(argv, list):
            argv = ["--enable-ldw-opt=true" if a == "--enable-ldw-opt=false"
                    else a for a in argv]
        return _orig_run_command(argv, **kwargs)

    _bu.run_command = _run_command_ldw

# model dims (fixed by the problem)
B, SQ, SKV, D, H = 32, 80, 1024, 1024, 16
L = int(os.environ.get("KERNEL_LAYERS", "12"))
HD = D // H          # 64
N_CORES = 8
BL = B // N_CORES    # 4 batch rows per core
DT = D // 128        # 8 d-tiles
FT = 4 * D // 128    # 32 ffn tiles
BQ = BL * SQ         # 320
EPS = 1e-5
SCALE = 1.0 / np.sqrt(HD)

F32 = mybir.dt.float32
F16 = mybir.dt.float16
BF16 = mybir.dt.bfloat16
FP8 = mybir.dt.float8e4
AF = mybir.ActivationFunctionType
ALU = mybir.AluOpType
FP8_SCALE = 64.0           # host prescales know + all weights into e4m3 range
FP8_INV = 1.0 / (FP8_SCALE * FP8_SCALE)
AV_SCALE = 4096.0          # aT carries 4096*a so fp8 stays in normal range
DR = mybir.MatmulPerfMode.DoubleRow


class PatchedTC(tile.TileContext):
    """This container's walrus accepts at most ONE sem wait per instruction;
    Tile may attach several. Peel extras onto preceding same-engine no-ops."""

    def _commit_instruction(self, inst, lazy_reg_writes: bool = True):
        si = getattr(inst, "sync_info", None)
        if (
            si is not None
            and si.on_wait
            and len(si.on_wait) > 1
            and inst.engine != mybir.EngineType.Unassigned
        ):
            waits = list(si.on_wait)
            si.on_wait = [waits[-1]]
            for j, w in enumerate(waits[:-1]):
                nop = mybir.InstNoOp(
                    name=f"{inst.name}-sw{j}",
                    sync_info=mybir.SyncInfo(on_wait=[w], on_update=[]),
                    bass_nofuse=True,
                    engine=inst.engine,
                )
                super()._commit_instruction(nop, lazy_reg_writes=False)
        return super()._commit_instruction(inst, lazy_reg_writes)

    def _drain_and_barrier(self, tick_clock, wait_clock):
        drain_inst = self.nc.sync.drain()
        wait_clock.add_sem_waits(
            drain_inst.ins, ScopedClock({None: tick_clock.global_clock})
        )
        si = drain_inst.ins.sync_info
        if si is not None and si.on_wait and len(si.on_wait) > 1:
            waits = list(si.on_wait)
            si.on_wait = waits[:1]
            for w in waits[1:]:
                extra = self.nc.sync.drain()
                nsi = extra.ins.sync_info
                if nsi is None:
                    extra.ins.sync_info = mybir.SyncInfo(on_wait=[w], on_update=[])
                else:
                    nsi.on_wait = [w]
        self.nc.all_engine_barrier()
        assert self.sems is not None
        popped = self.nc._tile_sem_poison_stack.pop()
        assert popped is self._sem_poison
        self.nc.clear_and_free_semaphores(list(self.sems.allocated().values()))
        self.nc.all_engine_barrier()


def bcast_ap(ap_1d, p):
    """Partition-broadcast a 1-D DRAM AP to [p, n] (stride-0 partition dim)."""
    return bass.AP(
        tensor=ap_1d.tensor, offset=ap_1d.offset, ap=[[0, p]] + list(ap_1d.ap)
    )


def build_nc():
    try:  # lift the stale 192KB/partition SBUF cap to the real usable 208KB
        from concourse import tile_utils

        tile_utils.max_sbuf_usage = 208 * 1024
    except Exception:
        pass

    nc = bass.Bass("TRN2", target_bir_lowering=False, debug=False,
                   num_devices=N_CORES)

    # ---- DRAM I/O (host-prepped layouts; see _prep() below) ----
    xT_in = nc.dram_tensor("xT0", [128, DT, BL, SQ], F16, kind="ExternalInput")
    knowT = nc.dram_tensor("knowT", [BL, 128, DT, SKV], FP8,
                           kind="ExternalInput")
    Wk = nc.dram_tensor("Wk", [L, DT, 128, DT, 128], FP8, kind="ExternalInput")
    Wv = nc.dram_tensor("Wv", [L, 128, DT, D], FP8, kind="ExternalInput")
    Wp = nc.dram_tensor("Wp", [L, DT, 128, DT, 128], FP8, kind="ExternalInput")
    Wf = nc.dram_tensor("Wf", [L, FT, 128, DT, 128], BF16, kind="ExternalInput")
    Wm = nc.dram_tensor("Wm", [L, DT, 128, FT, 128], BF16, kind="ExternalInput")
    bk = nc.dram_tensor("bk", [L, 128, DT], F32, kind="ExternalInput")
    bv = nc.dram_tensor("bv", [L, D], BF16, kind="ExternalInput")
    bp = nc.dram_tensor("bp", [L, 128, DT], F32, kind="ExternalInput")
    bf = nc.dram_tensor("bf", [L, 128, FT], F32, kind="ExternalInput")
    bm = nc.dram_tensor("bm", [L, 128, DT], F32, kind="ExternalInput")
    g1 = nc.dram_tensor("g1", [L, 128, DT], F32, kind="ExternalInput")
    b1 = nc.dram_tensor("b1", [L, 128, DT], F32, kind="ExternalInput")
    g2 = nc.dram_tensor("g2", [L, 128, DT], F32, kind="ExternalInput")
    b2 = nc.dram_tensor("b2", [L, 128, DT], F32, kind="ExternalInput")
    out_ext = nc.dram_tensor("out", [128, DT, BL, SQ], F16,
                             kind="ExternalOutput")

    with PatchedTC(nc) as tc:
        import contextlib

        ctx = contextlib.ExitStack()
        with ctx:
            P = lambda **kw: ctx.enter_context(tc.tile_pool(**kw))
            singles = P(name="singles", bufs=1)
            kv_pool = P(name="kv", bufs=2)       # kT_b + vb
            wT_pool = P(name="wT", bufs=2)
            wkv_pool = P(name="wkv", bufs=2)
            wch_pool = P(name="wch", bufs=4)     # wf chunks
            wp_pool = P(name="wpch", bufs=2)     # wp chunks
            wm_pool = P(name="wm", bufs=3)       # wm chunks (bigger)
            bc_pool = P(name="bc", bufs=2)       # broadcast tiles
            sb_pool = P(name="sb", bufs=2)       # per-layer small biases
            stA_pool = P(name="stA", bufs=1)     # LN tiny stats
            stB_pool = P(name="stB", bufs=2)     # softmax recip tiles
            sq_pool = P(name="sq", bufs=2)       # x^2 / LN scratch
            psA = P(name="psA", bufs=3, space="PSUM")  # [128,512] kv/proj/fc/mlp
            psS = P(name="psS", bufs=2, space="PSUM")  # [128,4,80] scoresT
            psV = P(name="psV", bufs=2, space="PSUM")  # [128,160] AV
            psM = P(name="psM", bufs=1, space="PSUM")  # [1,*] sums/LN stats
            drs = P(name="drs", bufs=4, space="DRAM")  # broadcast bounce bufs

            # ---- constants ----
            ones_f16 = singles.tile([128, 1], F16)
            nc.vector.memset(ones_f16, 1.0)
            # DR pair-ones carrying the 1/64 softmax-sum prescale; padded to
            # 16 cols so the pair stride is 16B (DR ldweights step%16==0).
            ones_i64 = singles.tile([128, 2, 16], FP8)
            nc.vector.memset(ones_i64, 1.0 / FP8_SCALE)
            eps_t = singles.tile([1, 1], F32)
            nc.vector.memset(eps_t, EPS)
            ones_1h = singles.tile([1, 16], F16)
            nc.vector.memset(ones_1h, 1.0)
            ones_1f = singles.tile([1, 16], F32)
            nc.vector.memset(ones_1f, 1.0)

            # ---- persistent activations ----
            xT = singles.tile([128, DT, BL, SQ], F16, tag="xT")
            nc.sync.dma_start(out=xT, in_=xT_in[:, :, :, :])
            know_res = singles.tile([128, BL, DT, SKV], FP8, tag="know")
            for kb in range(BL):
                nc.sync.dma_start(out=know_res[:, kb], in_=knowT[kb])
            # hbf holds the bf16 cast of LN1 out (MLP input); hq8 the fp8
            # cast of the residual (scores q input).
            hbf = singles.tile([128, DT, BL, SQ], BF16, tag="hbf")
            hq8 = singles.tile([128, DT, BL, SQ], FP8, tag="hq8")
            for dt in range(DT):
                nc.vector.tensor_copy(out=hq8[:, dt], in_=xT[:, dt])

            aT = singles.tile([128, DT, BL, SQ], FP8, tag="aT")
            gT = singles.tile([128, FT, BL, SQ], BF16, tag="gT")

            def ln_stats():
                """LN stats over the partition(d) axis of xT -> broadcast
                tile bc [128, 2, BQ] fp16 with bc[:,0]=mu, bc[:,1]=rstd."""
                ps_s = psM.tile([1, BQ], F32, tag="psM", name="ps_s")
                ps_q = psS.tile([1, BQ], F32, tag="psS", name="ps_q")
                for dt in range(DT):
                    x2 = xT[:, dt].rearrange("p b q -> p (b q)")
                    xsq = sq_pool.tile([128, BQ], F16, tag="lns", name="xsq")
                    nc.vector.tensor_tensor(xsq, x2, x2, ALU.mult)
                    nc.tensor.matmul(
                        ps_s, lhsT=ones_f16, rhs=x2,
                        start=(dt == 0), stop=(dt == DT - 1))
                    nc.tensor.matmul(
                        ps_q, lhsT=ones_f16, rhs=xsq,
                        start=(dt == 0), stop=(dt == DT - 1))
                stats = stA_pool.tile([1, 2, BQ], F16, tag="stats")
                # mu = ps_s/D ; musq = (ps_s/D)^2 ; var = ps_q/D - musq
                nc.vector.tensor_scalar_mul(stats[:, 0], ps_s, 1.0 / D)
                musq = stA_pool.tile([1, BQ], F32, tag="musq")
                nc.scalar.activation(musq, ps_s, AF.Square, scale=1.0 / D)
                var = stA_pool.tile([1, BQ], F32, tag="var")
                nc.vector.scalar_tensor_tensor(
                    out=var, in0=ps_q, scalar=1.0 / D, in1=musq,
                    op0=ALU.mult, op1=ALU.subtract)
                # rstd = exp(-0.5*ln(var+eps))  (Reciprocal/Rsqrt LUTs are
                # unavailable in this container's walrus)
                lnv = stA_pool.tile([1, BQ], F32, tag="lnv")
                nc.scalar.activation(lnv, var, AF.Ln, bias=eps_t)
                nc.scalar.activation(stats[:, 1], lnv, AF.Exp, scale=-0.5)
                # partition-broadcast via DRAM bounce (stride-0 read-back)
                dt_b = drs.tile([1, 2 * BQ], F16, tag="lnd")
                # write+read on the Pool DMA queue: 25ns dispatch (vs 565 on
                # sync) and same-FIFO ordering shortens the bounce chain
                nc.gpsimd.dma_start(
                    out=dt_b[0], in_=stats.rearrange("p a q -> p (a q)"))
                bc = bc_pool.tile([128, 2, BQ], F16, tag="lnbc")
                nc.gpsimd.dma_start(
                    out=bc.rearrange("p a q -> p (a q)"),
                    in_=bcast_ap(dt_b[0], 128))
                # PE keep-warm ticks pegged to the serial chain's stages so
                # the HAM window doesn't re-throttle during the stats gap
                for t, lh in ((var[:, 0:64], ones_1f[:, 0:1]),
                              (lnv[:, 0:64], ones_1f[:, 0:1]),
                              (stats[:, 0, 0:64], ones_1h[:, 0:1]),
                              (bc[0:1, 0, 0:64], ones_1h[:, 0:1])):
                    pw = psM.tile([1, 64], F32, tag="psM", name="warm")
                    nc.tensor.matmul(pw, lhsT=lh, rhs=t, start=True, stop=True)
                return bc

            def ln_apply(bc, g_sb, b_sb, cast_out):
                """x = (x - mu)*rstd*g + b ; cast_out = lowprec(x)."""
                for dt in range(DT):
                    x2 = xT[:, dt].rearrange("p b q -> p (b q)")
                    t = sq_pool.tile([128, BQ], F16, tag="lns", name="lnt")
                    nc.vector.tensor_tensor(t, x2, bc[:, 0], ALU.subtract)
                    nc.vector.tensor_tensor(t, t, bc[:, 1], ALU.mult)
                    nc.vector.tensor_scalar(
                        x2, t, g_sb[:, dt:dt + 1], b_sb[:, dt:dt + 1],
                        op0=ALU.mult, op1=ALU.add)
                    h2 = cast_out[:, dt].rearrange("p b q -> p (b q)")
                    if dt % 2 == 0:
                        nc.scalar.copy(out=h2, in_=x2)
                    else:
                        nc.vector.tensor_copy(out=h2, in_=x2)
                    if dt == 0:
                        # one keep-warm tick, ready exactly when the first
                        # consumer matmuls are (apply dt=0) — bridges the
                        # bounce->apply HAM hole without delaying them
                        pw = psM.tile([1, 64], F32, tag="psM", name="warm2")
                        nc.tensor.matmul(pw, lhsT=ones_f16, rhs=x2[:, 0:64],
                                         start=True, stop=True)

            def emit_layer_weights(l):
                """DMA layer-l kv weights + biases; returns handle dict."""
                w = {}
                w["wk"] = wkv_pool.tile([128, DT, DT, 128], FP8, tag="wk", name="wk")
                nc.sync.dma_start(
                    out=w["wk"], in_=Wk[l].rearrange("t p d n -> p t d n"))
                w["wv"] = wkv_pool.tile([128, DT, D], FP8, tag="wv", name="wv")
                nc.sync.dma_start(out=w["wv"], in_=Wv[l])
                for nm, src in [("bk", bk), ("bp", bp), ("bm", bm),
                                ("g1", g1), ("b1", b1), ("g2", g2),
                                ("b2", b2)]:
                    w[nm] = sb_pool.tile([128, DT], F32, tag=nm, name=nm)
                    nc.sync.dma_start(out=w[nm], in_=src[l])
                w["bf"] = sb_pool.tile([128, FT], F32, tag="bf", name="bfs")
                nc.sync.dma_start(out=w["bf"], in_=bf[l])
                w["bv"] = bc_pool.tile([128, D], BF16, tag="bv", name="bv")
                nc.gpsimd.dma_start(out=w["bv"], in_=bcast_ap(bv[l], 128))
                return w

            def emit_kT(w, b, nt_lo=0, nt_hi=DT, kTb=None):
                """K^T [n-part, s] for one b. k2-outer/sc-inner so the two
                MMs of each k2 share lhsT (amortizes LDWEIGHTS)."""
                if kTb is None:
                    kTb = kv_pool.tile([128, DT, SKV], FP8, tag="kT")
                for nt in range(nt_lo, nt_hi):
                    ps0 = psA.tile([128, 512], F32, tag="psA", name="psk0")
                    ps1 = psA.tile([128, 512], F32, tag="psA", name="psk1")
                    pss = (ps0, ps1)
                    for k2 in range(DT // 2):
                        for sc in range(2):
                            nc.tensor.matmul(
                                pss[sc],
                                lhsT=w["wk"][:, nt, 2 * k2:2 * k2 + 2, :],
                                rhs=know_res[:, b, 2 * k2:2 * k2 + 2,
                                             sc * 512:(sc + 1) * 512],
                                start=(k2 == 0), stop=(k2 == DT // 2 - 1),
                                perf_mode=DR)
                    # both drains on DVE: ACT is exp-saturated during the
                    # attention phase and would hold the psA slots hostage
                    for sc in range(2):
                        nc.vector.tensor_scalar(
                            kTb[:, nt, sc * 512:(sc + 1) * 512], pss[sc],
                            1.0 / FP8_SCALE, w["bk"][:, nt:nt + 1],
                            op0=ALU.mult, op1=ALU.add)
                return kTb

            def emit_V(w, b):
                """V [s-part, n] for one b. k2-outer/nh-inner so the two MMs
                of each k2 share lhsT (know chunk)."""
                vb = kv_pool.tile([128, DT, D], FP8, tag="v")
                for sv in range(DT):
                    ps0 = psA.tile([128, 512], F32, tag="psA", name="psv0")
                    ps1 = psA.tile([128, 512], F32, tag="psA", name="psv1")
                    pss = (ps0, ps1)
                    for k2 in range(DT // 2):
                        for nh in range(2):
                            nc.tensor.matmul(
                                pss[nh],
                                lhsT=know_res[:, b, 2 * k2:2 * k2 + 2,
                                              sv * 128:(sv + 1) * 128],
                                rhs=w["wv"][:, 2 * k2:2 * k2 + 2,
                                            nh * 512:(nh + 1) * 512],
                                start=(k2 == 0), stop=(k2 == DT // 2 - 1),
                                perf_mode=DR)
                    for nh in range(2):
                        nc.vector.scalar_tensor_tensor(
                            out=vb[:, sv, nh * 512:(nh + 1) * 512],
                            in0=pss[nh], scalar=1.0 / FP8_SCALE,
                            in1=w["bv"][:, nh * 512:(nh + 1) * 512],
                            op0=ALU.mult, op1=ALU.add)
                return vb

            def emit_scores(b, kTb):
                """scores^T -> exp into wT_b [s-part, sc, h, q]."""
                wTb = wT_pool.tile([128, DT, H, SQ], FP8, tag="wT")
                for h in range(H):
                    po = (h % 2) * 64
                    hp = h // 2
                    for g in range(2):
                        ps = psS.tile([128, 4, SQ], F32, tag="psS", name="pss")
                        for j in range(4):
                            sc = g * 4 + j
                            nc.tensor.matmul(
                                ps[:, j, :],
                                lhsT=kTb[po:po + 64, hp,
                                         sc * 128:(sc + 1) * 128],
                                rhs=hq8[po:po + 64, hp, b, :],
                                start=True, stop=True)
                        nc.scalar.activation(
                            out=wTb[:, g * 4:(g + 1) * 4, h, :],
                            in_=ps, func=AF.Exp, scale=SCALE / FP8_SCALE)
                return wTb

            def emit_sums(b, wTb):
                """softmax sums via DR ones(1/64) -> 64/sum via DVE fast
                reciprocal -> GpSimd partition-broadcast into rs_sb (f32)."""
                rs_sb = bc_pool.tile([128, H, SQ], F16, tag="rssb")
                for hg in range(4):
                    if hg % 2 == 0:
                        ps = psM.tile([1, 4 * SQ], F32, tag="psM",
                                      name="pssum")
                    else:
                        ps = psS.tile([1, 4 * SQ], F32, tag="psS",
                                      name="pssum2")
                    for g in range(4):
                        nc.tensor.matmul(
                            ps, lhsT=ones_i64[:, :, 0:1],
                            rhs=wTb[:, 2 * g:2 * g + 2, hg * 4:(hg + 1) * 4, :]
                            .rearrange("p k h q -> p k (h q)"),
                            start=(g == 0), stop=(g == 3),
                            perf_mode=DR)
                    # rs = 1/ps = 64/sum (ones carry 1/64): exp(-ln(ps))
                    lnp = stB_pool.tile([1, 4 * SQ], F32, tag="lnp")
                    nc.scalar.activation(lnp, ps, AF.Ln)
                    rp = stB_pool.tile([1, 4 * SQ], F16, tag="rs")
                    nc.scalar.activation(rp, lnp, AF.Exp, scale=-1.0)
                    dt_b = drs.tile([1, 4 * SQ], F16, tag="rsd")
                    nc.gpsimd.dma_start(out=dt_b[0], in_=rp)
                    nc.gpsimd.dma_start(
                        out=rs_sb[:, hg * 4:(hg + 1) * 4, :]
                        .rearrange("p h q -> p (h q)"),
                        in_=bcast_ap(dt_b[0], 128))
                return rs_sb

            def emit_AV(b, wTb, vb, rs_sb):
                """AV (head pairs) + normalize into aT[:, :, b, :]."""
                for hp in range(DT):
                    ps = psV.tile([128, 2 * SQ], F32, tag="psV", name="psav")
                    for sv in range(DT):
                        nc.tensor.matmul(
                            ps,
                            lhsT=vb[:, sv, hp * 128:(hp + 1) * 128],
                            rhs=wTb[:, sv, 2 * hp:2 * hp + 2, :].rearrange(
                                "p h q -> p (h q)"),
                            start=(sv == 0), stop=(sv == DT - 1))
                    nc.vector.tensor_tensor(
                        aT[0:64, hp, b, :], ps[0:64, 0:SQ],
                        rs_sb[0:64, 2 * hp, :], ALU.mult)
                    nc.vector.tensor_tensor(
                        aT[64:128, hp, b, :], ps[64:128, SQ:2 * SQ],
                        rs_sb[64:128, 2 * hp + 1, :], ALU.mult)

            # ================= layers (kv software-pipelined) =================
            # Emission order keeps independent matmul work queued ahead of
            # dependent ones (PE queue is strict FIFO: a waiting MM blocks
            # ready MMs behind it): V(b+1) lands before sums(b) so exp(b) has
            # drained; scores(b+1) lands before AV(b) so the rs broadcast
            # bounce has landed.
            w = emit_layer_weights(0)
            kT_t = {0: emit_kT(w, 0)}
            v_t = {0: emit_V(w, 0)}
            for l in range(L):
                wn = emit_layer_weights(l + 1) if l + 1 < L else None
                wTb = {0: emit_scores(0, kT_t[0])}
                if 1 < BL:
                    kT_t[1] = emit_kT(w, 1)
                kTb_n = None
                for b in range(BL):
                    if b + 1 < BL:
                        v_t[b + 1] = emit_V(w, b + 1)
                    rs_sb = emit_sums(b, wTb[b])
                    if b + 1 < BL:
                        wTb[b + 1] = emit_scores(b + 1, kT_t[b + 1])
                    if b + 2 < BL:
                        kT_t[b + 2] = emit_kT(w, b + 2)
                    elif b == BL - 1 and wn is not None:
                        # fills the sums(3) -> rs-bounce -> AV(3) latency
                        kTb_n = emit_kT(wn, 0, 0, DT // 2)
                    emit_AV(b, wTb[b], v_t[b], rs_sb)

                # 5-deep psum rotation for the dense-GEMM phases: psA's 3
                # slots plus psS's 2 (idle outside attention) so a group's
                # drain lag never stalls the next group's matmuls.
                def mm_ps(i, name):
                    j = i % 7
                    if j < 3:
                        return psA.tile([128, 512], F32, tag="psA", name=name)
                    if j < 5:
                        return psS.tile([128, 512], F32, tag="psS", name=name)
                    return psV.tile([128, 512], F32, tag="psV", name=name)

                # ---- attention out-projection + residual (fused drain) ----
                for nt in range(DT):
                    wpc = wp_pool.tile([128, DT, 128], FP8, tag="wp")
                    nc.sync.dma_start(out=wpc, in_=Wp[l, nt])
                    ps = mm_ps(nt, "psp")
                    for k2 in range(DT // 2):
                        nc.tensor.matmul(
                            ps[:, :BQ], lhsT=wpc[:, 2 * k2:2 * k2 + 2, :],
                            rhs=aT[:, 2 * k2:2 * k2 + 2].rearrange(
                                "p d b q -> p d (b q)"),
                            start=(k2 == 0), stop=(k2 == DT // 2 - 1),
                            perf_mode=DR)
                    x2 = xT[:, nt].rearrange("p b q -> p (b q)")
                    t = sq_pool.tile([128, BQ], F16, tag="lns", name="prt")
                    nc.vector.tensor_scalar(
                        t, ps[:, :BQ], 1.0 / (AV_SCALE * FP8_SCALE),
                        w["bp"][:, nt:nt + 1], op0=ALU.mult, op1=ALU.add)
                    nc.vector.tensor_tensor(x2, x2, t, ALU.add)

                # ---- LN1 stats, then next layer's V (fills the gap) ----
                bc1 = ln_stats()
                if l + 1 < L:
                    vb_n = emit_V(wn, 0)
                g1s, b1s, g2s, b2s, bfs = (w["g1"], w["b1"], w["g2"],
                                           w["b2"], w["bf"])
                ln_apply(bc1, g1s, b1s, hbf)

                # ---- ffn in + gelu ----
                for nt in range(FT):
                    wfc = wch_pool.tile([128, DT, 128], BF16, tag="wf")
                    nc.sync.dma_start(out=wfc, in_=Wf[l, nt])
                    ps = mm_ps(nt, "psf")
                    for kt in range(DT):
                        nc.tensor.matmul(
                            ps[:, :BQ], lhsT=wfc[:, kt],
                            rhs=hbf[:, kt].rearrange("p b q -> p (b q)"),
                            start=(kt == 0), stop=(kt == DT - 1))
                    nc.scalar.activation(
                        out=gT[:, nt].rearrange("p b q -> p (b q)"),
                        in_=ps[:, :BQ], func=AF.Gelu_apprx_tanh,
                        bias=bfs[:, nt:nt + 1])

                # ---- ffn out + residual ----
                for nt in range(DT):
                    wmc = wm_pool.tile([128, FT, 128], BF16, tag="wm")
                    nc.sync.dma_start(out=wmc, in_=Wm[l, nt])
                    ps = mm_ps(nt, "psm")
                    for kt in range(FT):
                        nc.tensor.matmul(
                            ps[:, :BQ], lhsT=wmc[:, kt],
                            rhs=gT[:, kt].rearrange("p b q -> p (b q)"),
                            start=(kt == 0), stop=(kt == FT - 1))
                    nc.vector.scalar_tensor_tensor(
                        out=xT[:, nt].rearrange("p b q -> p (b q)"),
                        in0=ps[:, :BQ], scalar=w["bm"][:, nt:nt + 1],
                        in1=xT[:, nt].rearrange("p b q -> p (b q)"),
                        op0=ALU.add, op1=ALU.add)

                # ---- LN2 stats; finish next layer's kT (fills the gap) ----
                bc2 = ln_stats()
                if l + 1 < L:
                    emit_kT(wn, 0, DT // 2, DT, kTb=kTb_n)
                    kT_t, v_t, w = {0: kTb_n}, {0: vb_n}, wn
                ln_apply(bc2, g2s, b2s, hq8)

            # epilogue: residual out (transposed; host un-transposes)
            nc.sync.dma_start(out=out_ext[:, :, :, :], in_=xT)

    return nc


_CACHE = {}


def _prep(inputs):
    """Host-side layout/dtype prep. Returns per-core in_maps."""
    import ml_dtypes

    bf16 = ml_dtypes.bfloat16
    fp8 = ml_dtypes.float8_e4m3
    f32 = np.float32

    x = np.asarray(inputs["input_ids"], f32) + np.asarray(
        inputs["pos_embed"], f32)[None]
    know = np.asarray(inputs["input_ids_know"], f32)
    Wa = np.asarray(inputs["W_attn"], f32)
    ba = np.asarray(inputs["b_attn"], f32)
    Wpm = np.asarray(inputs["W_proj_attn"], f32)
    bpm = np.asarray(inputs["b_proj_attn"], f32)
    Wfm = np.asarray(inputs["W_fc"], f32)
    bfm = np.asarray(inputs["b_fc"], f32)
    Wmm = np.asarray(inputs["W_proj_mlp"], f32)
    bmm = np.asarray(inputs["b_proj_mlp"], f32)

    def pt(a):  # [L, D'] -> [L, 128, T] with element [l, p, t] = a[l, t*128+p]
        return np.ascontiguousarray(
            a.reshape(L, -1, 128).transpose(0, 2, 1), f32)

    shared = {
        # Wk[l, nt, p, dt, n] = Wa[l, dt*128+p, D + nt*128+n]  (x64, fp8)
        "Wk": np.ascontiguousarray(
            (Wa[:L, :, D:2 * D] * 64.0).reshape(L, DT, 128, DT, 128)
            .transpose(0, 3, 2, 1, 4).astype(fp8)),
        # Wv[l, p, dt, n] = Wa[l, dt*128+p, 2D + n]  (x64, fp8)
        "Wv": np.ascontiguousarray(
            (Wa[:L, :, 2 * D:] * 64.0).reshape(L, DT, 128, D)
            .transpose(0, 2, 1, 3).astype(fp8)),
        # Wp[l, nt, p, kt, n] = Wp[l, kt*128+p, nt*128+n]  (x64, fp8)
        "Wp": np.ascontiguousarray(
            (Wpm[:L] * 64.0).reshape(L, DT, 128, DT, 128)
            .transpose(0, 3, 2, 1, 4).astype(fp8)),
        "Wf": np.ascontiguousarray(
            Wfm[:L].reshape(L, DT, 128, FT, 128)
            .transpose(0, 3, 2, 1, 4).astype(bf16)),
        "Wm": np.ascontiguousarray(
            Wmm[:L].reshape(L, FT, 128, DT, 128)
            .transpose(0, 3, 2, 1, 4).astype(bf16)),
        "bk": pt(ba[:L, D:2 * D] * 64.0),
        "bv": np.ascontiguousarray((ba[:L, 2 * D:] * 64.0).astype(bf16)),
        "bp": pt(bpm[:L]),
        "bf": pt(bfm[:L]),
        "bm": pt(bmm[:L]),
        "g1": pt(np.asarray(inputs["ln1_g"], f32)[:L]),
        "b1": pt(np.asarray(inputs["ln1_b"], f32)[:L]),
        "g2": pt(np.asarray(inputs["ln2_g"], f32)[:L]),
        "b2": pt(np.asarray(inputs["ln2_b"], f32)[:L]),
    }

    in_maps = []
    for c in range(N_CORES):
        m = dict(shared)
        xs = x[c * BL:(c + 1) * BL]  # [BL, SQ, D]
        # xT[p, dt, b, q] = xs[b, q, dt*128+p]
        m["xT0"] = np.ascontiguousarray(
            xs.reshape(BL, SQ, DT, 128).transpose(3, 2, 0, 1)).astype(
                np.float16)
        ks = know[c * BL:(c + 1) * BL]  # [BL, SKV, D]
        # knowT[b, p, dt, s] = ks[b, s, dt*128+p]
        m["knowT"] = np.ascontiguousarray(
            (ks * 64.0).reshape(BL, SKV, DT, 128)
            .transpose(0, 3, 2, 1).astype(fp8))
        in_maps.append(m)
    return in_maps


def kernel(**inputs):
    if "nc" not in _CACHE:
        _CACHE["nc"] = build_nc()
    nc = _CACHE["nc"]

    in_maps = _prep(inputs)
    _CACHE["last_in_maps"] = in_maps

    res = run_bass_kernel_spmd(nc, in_maps, list(range(N_CORES)))
    outs = []
    for c in range(N_CORES):
        oT = np.asarray(res.results[c]["out"]).astype(np.float32)
        # out[b, q, dt*128+p] = oT[p, dt, b, q]
        outs.append(oT.transpose(2, 3, 1, 0).reshape(BL, SQ, D))
    return np.ascontiguousarray(np.concatenate(outs, axis=0), np.float32)
